# revision 46
# baseline (speedup 1.0000x reference)
"""Trainium2 Bass kernel for CaPa-MoE (without clinical) MIL model.

Strategy (8 NeuronCores, SPMD, no collectives):
  - Shard the instance dim N=30000 -> 3750 rows/core.
  - Host pre-transposes + bf16-casts the big inputs so the device GEMM chain
    runs feature-major (features on partitions, instances on the free dim).
  - Per core the device computes: hv = h_v @ Wp + bp, hp = relu(hv @ W1 + b1),
    gated attention A = (tanh(hp@Wa+ba) * sigmoid(hp@Wb+bb)) @ Wc, e = exp(A),
    and the pooled partials Mp = e^T @ hp (per chunk) for both branches.
  - Host gathers: A (raw attention logits, + Wc bias), softmax normalizer
    s = sum(exp(A)), M = sum(Mp)/s, then the tiny [2,512]-scale experts /
    gate / classifier heads in float64.
  Per-class attention bias (vbc/ubc) cancels in softmax-over-instances, so the
  device pools with exp(A - bias-free); the bias is re-added on host for A_raw.
"""

import numpy as np
import ml_dtypes

import concourse.bacc as bacc
import concourse.bass as bass
import concourse.mybir as mybir
import concourse.tile as tile
from concourse import bass_utils, masks
from concourse._compat import get_trn_type

BF16 = mybir.dt.bfloat16
F32 = mybir.dt.float32
AF = mybir.ActivationFunctionType
MS = bass.MemorySpace

N_CORES = 8
N_TOTAL = 30000
N_LOC = N_TOTAL // N_CORES  # 3750
CH = 512  # instance-dim chunk width (matmul free dim)

_BF = ml_dtypes.bfloat16


def _chunks(n, ch):
    out, o = [], 0
    while o < n:
        w = min(ch, n - o)
        out.append((o, w))
        o += w
    return out


def _subtiles(w):
    return [(j * 128, min(128, w - j * 128)) for j in range((w + 127) // 128)]


def build(n_loc=N_LOC):
    nc = bacc.Bacc(
        get_trn_type() or "TRN2",
        target_bir_lowering=False,
        debug=False,
        enable_asserts=False,
    )
    chunks = _chunks(n_loc, CH)
    nch = len(chunks)
    max_nsub = max(len(_subtiles(w)) for _, w in chunks)

    di = lambda name, shape, dt=BF16: nc.dram_tensor(
        name, shape, dt, kind="ExternalInput"
    )
    do = lambda name, shape: nc.dram_tensor(name, shape, F32, kind="ExternalOutput")

    # all big tensors pre-packed on host to [128 partitions, k_tiles, cols]
    d_hvt = di("hvt", [128, 20, n_loc])
    d_hut = di("hut", [128, 8, n_loc])
    d_wp = di("wp", [128, 20, 1024])
    d_vw1 = di("vw1", [128, 8, 512])
    d_vwa = di("vwa", [128, 4, 256])
    d_vwb = di("vwb", [128, 4, 256])
    d_vwc = di("vwc", [128, 2, 2])
    d_uw1 = di("uw1", [128, 8, 512])
    d_uwa = di("uwa", [128, 4, 256])
    d_uwb = di("uwb", [128, 4, 256])
    d_uwc = di("uwc", [128, 2, 2])
    # [bp(8) vb1(4) vba(2) vbb/2(2) ub1(4) uba(2) ubb/2(2)]
    d_bias = di("bias", [128, 24], F32)

    d_av = do("a_v", [n_loc, 2])
    d_au = do("a_u", [n_loc, 2])
    d_mpv = do("mp_v", [nch, 2, 512])
    d_mpu = do("mp_u", [nch, 2, 512])

    with tile.TileContext(nc) as tc:
        with (
            tc.tile_pool(name="wt", bufs=1) as wt,
            tc.tile_pool(name="io", bufs=2) as io,
            tc.tile_pool(name="act", bufs=2) as actp,
            tc.tile_pool(name="sm", bufs=2) as smp,
            tc.tile_pool(name="pmm", bufs=6, space=MS.PSUM) as pmm,
            tc.tile_pool(name="psm", bufs=2, space=MS.PSUM) as psm,
        ):
            def load_w(d, k_tiles, cols, tag, kg=4):
                t = wt.tile([128, k_tiles, cols], BF16, tag=tag)
                for k0 in range(0, k_tiles, kg):
                    k1 = min(k0 + kg, k_tiles)
                    nc.sync.dma_start(t[:, k0:k1, :], d[:, k0:k1, :])
                return t

            io_hvt = [None] * nch
            io_hut = [None] * nch

            def load_chunk(c):
                n0, w = chunks[c]
                if io_hvt[c] is None:
                    hvt_c = io.tile([128, 20, CH], BF16, tag="hvt")
                    kgs = [1, 1, 3, 5, 5, 5] if c == 0 else [5, 5, 5, 5]
                    k0 = 0
                    for kg in kgs:
                        nc.sync.dma_start(
                            hvt_c[:, k0 : k0 + kg, :w],
                            d_hvt[:, k0 : k0 + kg, n0 : n0 + w],
                        )
                        k0 += kg
                    io_hvt[c] = hvt_c
                if io_hut[c] is None:
                    hut_c = io.tile([128, 8, CH], BF16, tag="hut")
                    for kg in range(2):
                        nc.sync.dma_start(
                            hut_c[:, kg * 4 : (kg + 1) * 4, :w],
                            d_hut[:, kg * 4 : (kg + 1) * 4, n0 : n0 + w],
                        )
                    io_hut[c] = hut_c

            # DMA emission order tracks first-use order; small leading wp
            # groups let the PE start within ~10us.
            wp_sb = wt.tile([128, 20, 1024], BF16, tag="wp")
            nc.sync.dma_start(wp_sb[:, 0, 0:128], d_wp[:, 0, 0:128])
            nc.sync.dma_start(wp_sb[:, 1, 0:128], d_wp[:, 1, 0:128])
            nc.sync.dma_start(wp_sb[:, 0, 128:1024], d_wp[:, 0, 128:1024])
            nc.sync.dma_start(wp_sb[:, 1, 128:1024], d_wp[:, 1, 128:1024])
            wp_groups = [(2, 3), (3, 4)] + [(k, k + 2) for k in range(4, 20, 2)]
            for k0, k1 in wp_groups[:3]:
                nc.sync.dma_start(wp_sb[:, k0:k1, :], d_wp[:, k0:k1, :])
            bias_sb = wt.tile([128, 24], F32, tag="bias")
            nc.sync.dma_start(bias_sb[:, :], d_bias[:, :])
            load_chunk(0)
            for k0, k1 in wp_groups[3:]:
                nc.sync.dma_start(wp_sb[:, k0:k1, :], d_wp[:, k0:k1, :])
            vw1_sb = load_w(d_vw1, 8, 512, "vw1")
            vwa_sb = load_w(d_vwa, 4, 256, "vwa")
            vwb_sb = load_w(d_vwb, 4, 256, "vwb")
            vwc_sb = load_w(d_vwc, 2, 2, "vwc")
            uw1_sb = load_w(d_uw1, 8, 512, "uw1")
            uwa_sb = load_w(d_uwa, 4, 256, "uwa")
            uwb_sb = load_w(d_uwb, 4, 256, "uwb")
            uwc_sb = load_w(d_uwc, 2, 2, "uwc")

            ident = wt.tile([128, 128], BF16, tag="ident")
            masks.make_identity(nc, ident[:])

            def branch(tag, x_c, w1_sb, b1o, wa_sb, bao, wb_sb, bbo,
                       wc_sb, d_a, d_mp, c, n0, w, subs):
                # hp^T = relu(x @ W1 + b1), feature-major [512, w]
                hpt = actp.tile([128, 4, CH], BF16, tag=f"hpt_{tag}")
                for m in range(4):
                    ps = pmm.tile([128, CH], F32, tag="mm")
                    for k in range(8):
                        nc.tensor.matmul(
                            ps[:, :w],
                            w1_sb[:, k, m * 128 : (m + 1) * 128],
                            x_c[:, k, :w],
                            start=(k == 0),
                            stop=(k == 7),
                        )
                    nc.scalar.activation(
                        hpt[:, m, :w], ps[:, :w], AF.Relu, bias=bias_sb[:, b1o + m : b1o + m + 1]
                    )

                # hp natural [n, 512] via PE transpose (for the e^T @ hp pool)
                hpn = actp.tile([128, max_nsub, 512], BF16, tag=f"hpn_{tag}")
                for j, (o, ns) in enumerate(subs):
                    pst = pmm.tile([128, 4, 128], BF16, tag="mm")
                    for m in range(4):
                        nc.tensor.transpose(
                            pst[0:ns, m, :], hpt[:, m, o : o + ns], ident[:]
                        )
                    for m in range(4):
                        nc.vector.tensor_copy(
                            hpn[0:ns, j, m * 128 : (m + 1) * 128],
                            pst[0:ns, m, :],
                        )

                # gated attention: g2 = tanh(hp@Wa+ba) * sigmoid(hp@Wb+bb)
                at = actp.tile([128, 2, CH], BF16, tag=f"at_{tag}")
                bt = actp.tile([128, 2, CH], BF16, tag=f"bt_{tag}")
                g2 = actp.tile([128, 2, CH], BF16, tag=f"g2_{tag}")
                for m in range(2):
                    psa = pmm.tile([128, CH], F32, tag="mm")
                    for k in range(4):
                        nc.tensor.matmul(
                            psa[:, :w],
                            wa_sb[:, k, m * 128 : (m + 1) * 128],
                            hpt[:, k, :w],
                            start=(k == 0),
                            stop=(k == 3),
                        )
                    nc.scalar.activation(
                        at[:, m, :w], psa[:, :w], AF.Tanh,
                        bias=bias_sb[:, bao + m : bao + m + 1],
                    )
                    psb = pmm.tile([128, CH], F32, tag="mm")
                    for k in range(4):
                        nc.tensor.matmul(
                            psb[:, :w],
                            wb_sb[:, k, m * 128 : (m + 1) * 128],
                            hpt[:, k, :w],
                            start=(k == 0),
                            stop=(k == 3),
                        )
                    # sigmoid(x) = 0.5*tanh(x/2) + 0.5 -- keeps every ACT call
                    # in the exp_and_others table set (no ~2.7us set switches).
                    # bb_sb holds the pre-halved bias.
                    nc.scalar.activation(
                        bt[:, m, :w], psb[:, :w], AF.Tanh,
                        bias=bias_sb[:, bbo + m : bbo + m + 1], scale=0.5,
                    )
                    nc.vector.tensor_scalar(
                        bt[:, m, :w], bt[:, m, :w], 0.5, 0.5,
                        mybir.AluOpType.mult, mybir.AluOpType.add,
                    )
                nc.vector.tensor_mul(g2[:, :, :w], at[:, :, :w], bt[:, :, :w])

                # A = g2 @ Wc (instance-major, [n, 2]); e = exp(A)
                pse = psm.tile([128, max_nsub, 2], F32, tag="ps")
                a_sb = smp.tile([128, max_nsub, 2], F32, tag=f"a_{tag}")
                e_sb = smp.tile([128, max_nsub, 2], BF16, tag=f"e_{tag}")
                for j, (o, ns) in enumerate(subs):
                    for k in range(2):
                        nc.tensor.matmul(
                            pse[0:ns, j, :],
                            g2[:, k, o : o + ns],
                            wc_sb[:, k, :],
                            start=(k == 0),
                            stop=(k == 1),
                        )
                    nc.scalar.activation(a_sb[0:ns, j, :], pse[0:ns, j, :], AF.Copy)
                    nc.scalar.activation(e_sb[0:ns, j, :], pse[0:ns, j, :], AF.Exp)
                    nc.sync.dma_start(
                        d_a[n0 + o : n0 + o + ns, :], a_sb[0:ns, j, :]
                    )

                # pooled partial Mp = e^T @ hp  [2, 512]
                psp = psm.tile([2, 512], F32, tag="ps")
                for j, (o, ns) in enumerate(subs):
                    nc.tensor.matmul(
                        psp[:, :],
                        e_sb[0:ns, j, :],
                        hpn[0:ns, j, :],
                        start=(j == 0),
                        stop=(j == len(subs) - 1),
                    )
                mp_sb = smp.tile([2, 512], F32, tag=f"mp_{tag}")
                nc.vector.tensor_copy(mp_sb[:, :], psp[:, :])
                nc.sync.dma_start(d_mp[c], mp_sb[:, :])

            for c, (n0, w) in enumerate(chunks):
                subs = _subtiles(w)

                load_chunk(c)
                hvt_c, hut_c = io_hvt[c], io_hut[c]

                # virchow projection: hv^T = Wp^T @ hv^T + bp  [1024, w]
                hv_c = actp.tile([128, 8, CH], BF16, tag="hv")
                for m in range(8):
                    ps = pmm.tile([128, CH], F32, tag="mm")
                    for k in range(20):
                        nc.tensor.matmul(
                            ps[:, :w],
                            wp_sb[:, k, m * 128 : (m + 1) * 128],
                            hvt_c[:, k, :w],
                            start=(k == 0),
                            stop=(k == 19),
                        )
                    nc.scalar.activation(
                        hv_c[:, m, :w], ps[:, :w], AF.Identity,
                        bias=bias_sb[:, m : m + 1],
                    )

                branch("u", hut_c, uw1_sb, 16, uwa_sb, 20, uwb_sb,
                       22, uwc_sb, d_au, d_mpu, c, n0, w, subs)
                branch("v", hv_c, vw1_sb, 8, vwa_sb, 12, vwb_sb,
                       14, vwc_sb, d_av, d_mpv, c, n0, w, subs)

    nc.compile()
    return nc


def _pack_w(x, k_tiles):
    """[K, F] -> [128, k_tiles, F] bf16 with row k*128+p at [p, k, :]."""
    x = np.asarray(x)
    return np.ascontiguousarray(
        x.reshape(k_tiles, 128, x.shape[1]).transpose(1, 0, 2)
    ).astype(_BF)


def make_in_maps(inputs, n_loc=N_LOC, n_cores=N_CORES):
    f = lambda x: np.asarray(x, dtype=np.float32)

    bias = np.zeros((128, 24), np.float32)
    bias[:, 0:8] = f(inputs["bp"]).reshape(8, 128).T
    bias[:, 8:12] = f(inputs["vb1"]).reshape(4, 128).T
    bias[:, 12:14] = f(inputs["vba"]).reshape(2, 128).T
    bias[:, 14:16] = 0.5 * f(inputs["vbb"]).reshape(2, 128).T
    bias[:, 16:20] = f(inputs["ub1"]).reshape(4, 128).T
    bias[:, 20:22] = f(inputs["uba"]).reshape(2, 128).T
    bias[:, 22:24] = 0.5 * f(inputs["ubb"]).reshape(2, 128).T

    shared = {
        "wp": _pack_w(inputs["Wp"], 20),
        "vw1": _pack_w(inputs["vW1"], 8),
        "vwa": _pack_w(inputs["vWa"], 4),
        "vwb": _pack_w(inputs["vWb"], 4),
        "vwc": _pack_w(inputs["vWc"], 2),
        "uw1": _pack_w(inputs["uW1"], 8),
        "uwa": _pack_w(inputs["uWa"], 4),
        "uwb": _pack_w(inputs["uWb"], 4),
        "uwc": _pack_w(inputs["uWc"], 2),
        "bias": bias,
    }
    h_v = np.asarray(inputs["h_virchow"])
    h_u = np.asarray(inputs["h_UNI"])
    in_maps = []
    for c in range(n_cores):
        sl = slice(c * n_loc, (c + 1) * n_loc)
        # [n, K] -> [128, k_tiles, n]: element (p, k, n) = x[n, k*128+p]
        hv = h_v[sl].T.astype(_BF, order="C").reshape(20, 128, n_loc)
        hu = h_u[sl].T.astype(_BF, order="C").reshape(8, 128, n_loc)
        m = dict(shared)
        m["hvt"] = np.ascontiguousarray(hv.transpose(1, 0, 2))
        m["hut"] = np.ascontiguousarray(hu.transpose(1, 0, 2))
        in_maps.append(m)
    return in_maps


def host_epilogue(inputs, A_v, A_u, Mp_v, Mp_u):
    """A_v/A_u: [N,2] f32 device attention logits (no Wc bias).
    Mp_v/Mp_u: [2,512] f64 summed pooled partials (exp-weighted hp sums)."""
    d = lambda k: np.asarray(inputs[k], dtype=np.float64)

    def relu(x):
        return np.maximum(x, 0.0)

    A_v64 = A_v.astype(np.float64)
    A_u64 = A_u.astype(np.float64)
    s_v = np.exp(A_v64).sum(axis=0)  # [2]
    s_u = np.exp(A_u64).sum(axis=0)
    M_v = Mp_v / s_v[:, None]  # [2, 512]
    M_u = Mp_u / s_u[:, None]

    A_v_raw = (A_v64 + d("vbc")[None, :]).T.astype(np.float32)  # [2, N]
    A_u_raw = (A_u64 + d("ubc")[None, :]).T.astype(np.float32)

    feat1 = relu(relu(M_v @ d("e1W1") + d("e1b1")) @ d("e1W2") + d("e1b2"))
    feat3 = relu(relu(M_u @ d("e3W1") + d("e3b1")) @ d("e3W2") + d("e3b2"))
    f2in = np.concatenate([M_v, M_u], axis=-1)
    feat2 = (
        relu(relu(f2in @ d("e2W1") + d("e2b1")) @ d("e2W2") + d("e2b2")) @ d("e2pW")
        + d("e2pb")
    )

    g = np.concatenate([M_v.mean(axis=0), M_u.mean(axis=0)], axis=-1)  # [1024]
    g_logits = relu(g @ d("gW1") + d("gb1")) @ d("gW2") + d("gb2")  # [3]
    ge = np.exp(g_logits - g_logits.max())
    p = ge / ge.sum()

    fused = p[0] * feat1 + p[1] * feat2 + p[2] * feat3  # [2, 512]
    logits = (np.einsum("ch,ch->c", fused, d("clsW")) + d("clsb"))[None, :]  # [1,2]
    le = np.exp(logits - logits.max(axis=1, keepdims=True))
    Y_prob = (le / le.sum(axis=1, keepdims=True)).astype(np.float32)
    Y_hat = np.argmax(logits, axis=1).astype(np.int32)
    return (
        logits.astype(np.float32),
        Y_prob,
        Y_hat,
        A_v_raw,
        A_u_raw,
    )


_CACHE = {}


def run(inputs, trace=False):
    if "nc" not in _CACHE:
        _CACHE["nc"] = build()
    nc = _CACHE["nc"]
    in_maps = make_in_maps(inputs)
    res = bass_utils.run_bass_kernel_spmd(
        nc, in_maps, core_ids=list(range(N_CORES)), trace=trace
    )
    outs = res.results
    A_v = np.concatenate([np.asarray(o["a_v"], np.float32) for o in outs], axis=0)
    A_u = np.concatenate([np.asarray(o["a_u"], np.float32) for o in outs], axis=0)
    Mp_v = sum(np.asarray(o["mp_v"], np.float64).sum(axis=0) for o in outs)
    Mp_u = sum(np.asarray(o["mp_u"], np.float64).sum(axis=0) for o in outs)
    return host_epilogue(inputs, A_v, A_u, Mp_v, Mp_u), res


def kernel(**inputs):
    out, _ = run(inputs, trace=False)
    return out
